# revision 13
# baseline (speedup 1.0000x reference)
"""Causal self-attention with RoPE on 8 Trainium2 NeuronCores (Bass/Tile).

Self-contained: builds an SPMD Bass kernel, shards the full inputs across the
8 cores as (batch b = core//2, head-half s = core%2), runs via PJRT, and
re-assembles the full [4, 2048, 1024] output (sum of the two proj partials
per batch).

v2: bf16 storage/matmuls (f32 PSUM), flipped PV matmuls (free dim 65),
DMA-engine transposes for the proj input, phase-1 chunks interleaved with
per-qi attention sections so exp (ACT engine) overlaps the GEMMs.

Problem shapes (hardcoded): B=4, T=2048, C=1024, H=16, D=64.
"""
import sys
sys.path.insert(0, "/opt/trn_rl_repo")
import numpy as np
from contextlib import ExitStack

import concourse.bass as bass
import concourse.bacc as bacc
import concourse.mybir as mybir
import concourse.tile as tile

F32 = mybir.dt.float32
BF16 = mybir.dt.bfloat16
AF = mybir.ActivationFunctionType

B = 4
T = 2048
N_EMBD = 1024
D = 64
HL = 8                      # local heads per core
CL = HL * D                 # 512
CT = CL // 128              # 4
VW = HL * 65                # 520 (64 + ones col per head)
CIN_TILES = N_EMBD // 128   # 8
N_CORES = 8
NQT = T // 512              # 4 q blocks (one per phase-1 chunk)
NKT = T // 128              # 16 kv tiles


# ====================== device program ======================

def _build_nc():
    nc = bacc.Bacc("TRN2", target_bir_lowering=False, debug=False,
                   num_devices=N_CORES)

    xT = nc.declare_dram_parameter("xT", [N_EMBD, T], BF16, isOutput=False)
    wq = nc.declare_dram_parameter("wq", [N_EMBD, CL], BF16, isOutput=False)
    wk = nc.declare_dram_parameter("wk", [N_EMBD, CL], BF16, isOutput=False)
    wv = nc.declare_dram_parameter("wv", [N_EMBD, VW], BF16, isOutput=False)
    projw = nc.declare_dram_parameter("projw", [CL, N_EMBD], BF16,
                                      isOutput=False)
    cosT = nc.declare_dram_parameter("cosT", [128, T], BF16, isOutput=False)
    sinS = nc.declare_dram_parameter("sinS", [128, T], BF16, isOutput=False)
    r2 = nc.declare_dram_parameter("r2", [128, 128], BF16, isOutput=False)
    qkbias = nc.declare_dram_parameter("qkbias", [128, 2 * CT], F32,
                                       isOutput=False)
    bv = nc.declare_dram_parameter("bv", [1, VW], BF16, isOutput=False)
    pbh = nc.declare_dram_parameter("pbh", [128, N_EMBD], BF16, isOutput=False)
    ones = nc.declare_dram_parameter("ones", [1, 128], BF16, isOutput=False)
    masks = nc.declare_dram_parameter("masks", [128, 1408], BF16,
                                      isOutput=False)
    out = nc.declare_dram_parameter("out", [T, N_EMBD], BF16, isOutput=True)

    with tile.TileContext(nc) as tc, ExitStack() as ctx:
        res = ctx.enter_context(tc.tile_pool(name="res", bufs=1))
        qt_tiles = [res.tile([128, T], BF16, tag=f"qt{i}", name=f"qt{i}")
                    for i in range(CT)]
        kt_tiles = [res.tile([128, T], BF16, tag=f"kt{i}", name=f"kt{i}")
                    for i in range(CT)]
        v_tiles = [res.tile([128, VW], BF16, tag=f"v{j}", name=f"v{j}")
                   for j in range(NKT)]
        yt = res.tile([128, CT * T], BF16, tag="yt")
        wq_sb = res.tile([128, CIN_TILES * CL], BF16, tag="wq")
        wk_sb = res.tile([128, CIN_TILES * CL], BF16, tag="wk")
        wv_sb = res.tile([128, CIN_TILES * VW], BF16, tag="wv")
        pw_sb = res.tile([128, CT * N_EMBD], BF16, tag="pw")
        cos_sb = res.tile([128, T], BF16, tag="cos")
        sin_sb = res.tile([128, T], BF16, tag="sin")
        qkb_sb = res.tile([128, 2 * CT], F32, tag="qkb")
        bvr_sb = res.tile([1, VW], BF16, tag="bvr")
        pbh_sb = res.tile([128, N_EMBD], BF16, tag="pbh")
        ones_sb = res.tile([1, 128], BF16, tag="ones")
        mask_sb = res.tile([128, 1408], BF16, tag="masks")
        r2_sb = res.tile([128, 128], BF16, tag="r2")

        # resident DMAs (weights split per cout tile so early deps land first)
        nc.sync.dma_start(ones_sb[:], ones[:])
        nc.sync.dma_start(qkb_sb[:], qkbias[:])
        nc.sync.dma_start(r2_sb[:], r2[:])
        for i in range(CT):
            nc.sync.dma_start(
                wq_sb.rearrange("p (a c) -> p a c", a=CIN_TILES)
                [:, :, i * 128:(i + 1) * 128],
                wq[:, i * 128:(i + 1) * 128]
                .rearrange("(a p) c -> p a c", p=128))
        for i in range(CT):
            nc.sync.dma_start(
                wk_sb.rearrange("p (a c) -> p a c", a=CIN_TILES)
                [:, :, i * 128:(i + 1) * 128],
                wk[:, i * 128:(i + 1) * 128]
                .rearrange("(a p) c -> p a c", p=128))
        nc.sync.dma_start(cos_sb[:], cosT[:])
        nc.sync.dma_start(sin_sb[:], sinS[:])
        nc.sync.dma_start(
            wv_sb.rearrange("p (a c) -> p a c", a=CIN_TILES),
            wv[:].rearrange("(a p) c -> p a c", p=128))
        nc.sync.dma_start(bvr_sb[:], bv[:])
        nc.sync.dma_start(mask_sb[:], masks[:])
        nc.sync.dma_start(
            pw_sb.rearrange("p (a c) -> p a c", a=CT),
            projw[:].rearrange("(a p) c -> p a c", p=128))
        nc.sync.dma_start(pbh_sb[:], pbh[:])

        xpool = ctx.enter_context(tc.tile_pool(name="xpool", bufs=2))
        qbpool = ctx.enter_context(tc.tile_pool(name="qbpool", bufs=3))
        t2pool = ctx.enter_context(tc.tile_pool(name="t2pool", bufs=3))
        ppool = ctx.enter_context(tc.tile_pool(name="ppool", bufs=10))
        ysbp = ctx.enter_context(tc.tile_pool(name="ysbp", bufs=8))
        recp = ctx.enter_context(tc.tile_pool(name="recp", bufs=3))
        otp = ctx.enter_context(tc.tile_pool(name="otp", bufs=4))
        mmps = ctx.enter_context(tc.tile_pool(name="mmps", bufs=2,
                                              space="PSUM"))
        sps = ctx.enter_context(tc.tile_pool(name="sps", bufs=2,
                                             space="PSUM"))
        yqps = ctx.enter_context(tc.tile_pool(name="yqps", bufs=1,
                                              space="PSUM"))
        rotps = ctx.enter_context(tc.tile_pool(name="rotps", bufs=1,
                                               space="PSUM"))

        xc_tiles = {}
        y_sb_cur = {}          # global q-tile idx -> y_sb tile

        def emit_xc(j):
            xc = xpool.tile([128, CIN_TILES * 512], BF16, tag="xc", name="xc")
            tj = slice(j * 512, (j + 1) * 512)
            nc.sync.dma_start(
                xc.rearrange("p (a t) -> p a t", a=CIN_TILES),
                xT[:, tj].rearrange("(a p) t -> p a t", p=128))
            xc_tiles[j] = xc

        def emit_qk_tile(j, which, i):
            """QKV projection + RoPE for one [128, 512] q/k tile."""
            tj = slice(j * 512, (j + 1) * 512)
            xc = xc_tiles[j]
            w_sb = wq_sb if which == 0 else wk_sb
            t_dst = qt_tiles if which == 0 else kt_tiles
            ps = mmps.tile([128, 512], F32, tag="mm", name="qkps")
            for a in range(CIN_TILES):
                nc.tensor.matmul(
                    ps[:],
                    w_sb[:, a * CL + i * 128:a * CL + (i + 1) * 128],
                    xc[:, a * 512:(a + 1) * 512],
                    start=(a == 0), stop=(a == CIN_TILES - 1))
            qb = qbpool.tile([128, 512], BF16, tag="qb", name="qb")
            bcol = which * CT + i
            nc.vector.tensor_scalar_add(qb[:], ps[:],
                                        qkb_sb[:, bcol:bcol + 1])
            dst = t_dst[i][:, tj]
            # rot(qb) via the PE permutation matrix (partition moves must go
            # through the PE; TensorTensor requires same start partitions)
            rps = rotps.tile([128, 512], F32, tag="rot", name="rot")
            nc.tensor.matmul(rps[:], r2_sb[:], qb[:], start=True, stop=True)
            # t1 = qb * cos (into destination; SBUF-only so Pool can do it)
            nc.gpsimd.tensor_mul(dst, qb[:], cos_sb[:, tj])
            t2 = t2pool.tile([128, 512], BF16, tag="t2", name="t2")
            nc.vector.tensor_mul(t2[:], rps[:], sin_sb[:, tj])
            nc.vector.tensor_add(dst, dst, t2[:])

        def emit_v_tile(j, tt):
            """V projection for kv tile j*4+tt ([128, 520], time-major)."""
            vt = v_tiles[j * 4 + tt]
            for ch in range(2):
                cw = VW // 2
                vsl = slice(ch * cw, (ch + 1) * cw)
                ps = mmps.tile([128, 512], F32, tag="mm", name="vps")
                for a in range(CIN_TILES):
                    nc.tensor.matmul(
                        ps[:, 0:cw],
                        xc_tiles[j][:, a * 512 + tt * 128:
                                    a * 512 + (tt + 1) * 128],
                        wv_sb[:, a * VW:(a + 1) * VW][:, vsl],
                        start=(a == 0), stop=False)
                nc.tensor.matmul(ps[:, 0:cw], ones_sb[:], bvr_sb[:, vsl],
                                 start=False, stop=True)
                nc.vector.tensor_copy(vt[:, vsl], ps[:, 0:cw])

        def emit_attn_head(qi, h):
            qs = slice(qi * 512, (qi + 1) * 512)
            nk = 4 * (qi + 1)
            th, pb = h // 2, (h % 2) * 64
            kt_h = kt_tiles[th]
            qt_h = qt_tiles[th]
            q0 = qi * 512
            pts = {}
            # ---- S + exp (+ mask) per k-tile pair ----
            for kp in range(nk // 2):
                kb0, kb1 = 2 * kp, 2 * kp + 1
                ks0 = slice(kb0 * 128, (kb0 + 1) * 128)
                ks1 = slice(kb1 * 128, (kb1 + 1) * 128)
                kt0 = kt_h[pb:pb + 64, ks0]
                kt1 = kt_h[pb:pb + 64, ks1]
                mp = kp - 2 * qi
                if mp < 0:                 # fully-causal pair
                    sp = sps.tile([128, 1024], F32, tag="sp", name="sp")
                    nc.tensor.matmul(sp[:, 0:512], kt0, qt_h[pb:pb + 64, qs],
                                     start=True, stop=True)
                    nc.tensor.matmul(sp[:, 512:1024], kt1,
                                     qt_h[pb:pb + 64, qs],
                                     start=True, stop=True)
                    pt = ppool.tile([128, 1024], BF16, tag="pt", name="pt")
                    nc.scalar.activation(pt[:], sp[:], AF.Exp, scale=0.125)
                elif mp == 0:              # first diagonal pair
                    sp = sps.tile([128, 1024], F32, tag="sp", name="sp")
                    nc.tensor.matmul(sp[:, 0:512], kt0, qt_h[pb:pb + 64, qs],
                                     start=True, stop=True)
                    nc.tensor.matmul(sp[:, 512:896], kt1,
                                     qt_h[pb:pb + 64, q0 + 128:q0 + 512],
                                     start=True, stop=True)
                    pt = ppool.tile([128, 1024], BF16, tag="pt", name="pt")
                    nc.scalar.activation(pt[:, 0:896], sp[:, 0:896], AF.Exp,
                                         scale=0.125)
                    nc.gpsimd.tensor_mul(pt[:, 0:896], pt[:, 0:896],
                                         mask_sb[:, 0:896])
                else:                      # last diagonal pair
                    qsub = qt_h[pb:pb + 64, q0 + 256:q0 + 512]
                    sp = sps.tile([128, 1024], F32, tag="sp", name="sp")
                    nc.tensor.matmul(sp[:, 0:256], kt0, qsub,
                                     start=True, stop=True)
                    nc.tensor.matmul(sp[:, 256:512], kt1, qsub,
                                     start=True, stop=True)
                    pt = ppool.tile([128, 1024], BF16, tag="pt", name="pt")
                    nc.scalar.activation(pt[:, 0:512], sp[:, 0:512], AF.Exp,
                                         scale=0.125)
                    nc.gpsimd.tensor_mul(pt[:, 0:512], pt[:, 0:512],
                                         mask_sb[:, 896:1408])
                pts[kp] = pt

            # pt column offset for (kb, qq): cols of q-subtile within pt
            def pt_cols(kb, qq):
                kp, half = kb // 2, kb % 2
                mp = kp - 2 * qi
                if mp < 0:
                    base = half * 512 + qq * 128
                elif mp == 0:
                    base = qq * 128 if half == 0 else 512 + (qq - 1) * 128
                else:
                    base = (qq - 2) * 128 if half == 0 else 256 + (qq - 2) * 128
                return pts[kp][:, base:base + 128]

            # ---- PV (flipped): yq[q, 65] += pt[kt, q]^T @ v[kt, 65] ----
            yq = yqps.tile([128, 4 * 65], F32, tag="yq", name="yq")
            vsl = slice(h * 65, h * 65 + 65)
            for qq in range(4):
                lo = 0
                hi = 4 * qi + qq            # last kv tile for this q-subtile
                for kb in range(lo, hi + 1):
                    nc.tensor.matmul(yq[:, qq * 65:(qq + 1) * 65],
                                     pt_cols(kb, qq),
                                     v_tiles[kb][:, vsl],
                                     start=(kb == lo), stop=(kb == hi))
            # ---- normalize rows into y_sb (q-major) ----
            rec = recp.tile([128, 4], F32, tag="rec", name="rec")
            for qq in range(4):
                nc.vector.reciprocal(rec[:, qq:qq + 1],
                                     yq[:, qq * 65 + 64:qq * 65 + 65])
            for qq in range(4):
                gq = qi * 4 + qq
                if gq not in y_sb_cur:
                    y_sb_cur[gq] = ysbp.tile([128, 512], BF16, tag="ysb",
                                             name=f"ysb{gq}")
                nc.vector.tensor_scalar_mul(
                    y_sb_cur[gq][:, h * 64:(h + 1) * 64],
                    yq[:, qq * 65:qq * 65 + 64], rec[:, qq:qq + 1])

        def emit_transposes(qi):
            """DMA-engine transposes: y_sb [q, d] -> yt [d, q] per d-chunk."""
            for qq in range(4):
                gq = qi * 4 + qq
                ysb = y_sb_cur[gq]
                for c in range(CT):
                    nc.sync.dma_start_transpose(
                        yt[:, c * T + gq * 128:c * T + (gq + 1) * 128],
                        ysb[:, c * 128:(c + 1) * 128])

        def emit_proj_tt(tt):
            for ch in range(2):
                cs = slice(ch * 512, (ch + 1) * 512)
                ps = mmps.tile([128, 512], F32, tag="mm", name="ops")
                for a in range(CT):
                    nc.tensor.matmul(
                        ps[:],
                        yt[:, a * T + tt * 128:a * T + (tt + 1) * 128],
                        pw_sb[:, a * N_EMBD:(a + 1) * N_EMBD][:, cs],
                        start=(a == 0), stop=(a == CT - 1))
                ot = otp.tile([128, 512], BF16, tag="ot", name="ot")
                nc.vector.tensor_add(ot[:], ps[:], pbh_sb[:, cs])
                nc.sync.dma_start(out[tt * 128:(tt + 1) * 128, cs], ot[:])

        # ================= schedule =================
        emit_xc(0)
        for which in range(2):
            for i in range(CT):
                emit_qk_tile(0, which, i)
        for tt in range(4):
            emit_v_tile(0, tt)

        # sections j=1..3: phase-1 chunk j interleaved with attention qi=j-1
        for j in range(1, 4):
            emit_xc(j)
            p1_units = ([(0, i) for i in range(CT)] +
                        [(1, i) for i in range(CT)])
            v_units = list(range(4))
            taken_p1 = taken_v = 0
            for h in range(HL):
                # spread 8 qk + 4 v units across the 8 heads
                want_p1 = ((h + 1) * 8 + 7) // 8
                while taken_p1 < min(want_p1, 8):
                    which, i = p1_units[taken_p1]
                    emit_qk_tile(j, which, i)
                    taken_p1 += 1
                want_v = ((h + 1) * 4) // 8
                while taken_v < want_v:
                    emit_v_tile(j, v_units[taken_v])
                    taken_v += 1
                emit_attn_head(j - 1, h)
            while taken_p1 < 8:
                which, i = p1_units[taken_p1]
                emit_qk_tile(j, which, i)
                taken_p1 += 1
            while taken_v < 4:
                emit_v_tile(j, v_units[taken_v])
                taken_v += 1
            emit_transposes(j - 1)

        # final section: attention qi=3 interleaved with proj of qi=0..2
        proj_units = list(range(12))        # tt 0..11
        taken_pr = 0
        for h in range(HL):
            emit_attn_head(3, h)
            want_pr = ((h + 1) * 12) // 8
            while taken_pr < want_pr:
                emit_proj_tt(proj_units[taken_pr])
                taken_pr += 1
        emit_transposes(3)
        for tt in range(12, 16):
            emit_proj_tt(tt)

    nc.compile()
    return nc


# ====================== host-side sharding ======================

def _rope_tables():
    inv_freq = 1.0 / (10000.0 ** (np.arange(0, D, 2, dtype=np.float64) / D))
    t = np.arange(T, dtype=np.float64)
    fr = np.outer(t, inv_freq)
    emb = np.concatenate([fr, fr], axis=-1)
    return np.cos(emb), np.sin(emb)


def _bf16(a):
    import ml_dtypes
    return np.ascontiguousarray(np.asarray(a).astype(ml_dtypes.bfloat16))


def _make_in_maps(x, qkv_w, qkv_b, proj_w, proj_b):
    C = N_EMBD
    cos, sin = _rope_tables()                   # [T, 64] each
    cos2 = np.vstack([cos.T, cos.T])            # [128, T]
    sin2 = np.vstack([sin.T, sin.T])            # [128, T]
    M = np.zeros((D, D), dtype=np.float32)
    for d in range(32):
        M[d, d + 32] = -1.0
        M[d + 32, d] = 1.0
    R2 = np.zeros((128, 128), dtype=np.float32)
    R2[:64, :64] = M.T
    R2[64:, 64:] = M.T

    p = np.arange(128)[:, None]

    def m_off(off, w):
        f = np.arange(w)[None, :]
        return ((p + off) <= f).astype(np.float32)

    msk = np.concatenate([m_off(0, 512), m_off(0, 384),
                          m_off(0, 256), m_off(128, 256)], axis=1)
    ones_np = np.ones((1, 128), dtype=np.float32)
    pbh_np = np.tile((proj_b / 2.0).astype(np.float32)[None, :], (128, 1))

    in_maps = []
    for c in range(N_CORES):
        b, s = divmod(c, 2)
        cl0 = s * CL
        wq_ = qkv_w[:, cl0:cl0 + CL]
        wk_ = qkv_w[:, C + cl0:C + cl0 + CL]
        wv_raw = qkv_w[:, 2 * C + cl0:2 * C + cl0 + CL]
        bq = qkv_b[cl0:cl0 + CL]
        bk = qkv_b[C + cl0:C + cl0 + CL]
        bv_raw = qkv_b[2 * C + cl0:2 * C + cl0 + CL]
        wv_ = np.zeros((C, VW), dtype=np.float32)
        bv_ = np.zeros((1, VW), dtype=np.float32)
        for h in range(HL):
            wv_[:, 65 * h:65 * h + 64] = wv_raw[:, 64 * h:64 * h + 64]
            bv_[0, 65 * h:65 * h + 64] = bv_raw[64 * h:64 * h + 64]
            bv_[0, 65 * h + 64] = 1.0
        qkb = np.zeros((128, 2 * CT), dtype=np.float32)
        for i in range(CT):
            qkb[:, i] = bq[i * 128:(i + 1) * 128]
            qkb[:, CT + i] = bk[i * 128:(i + 1) * 128]
        in_maps.append({
            "xT": _bf16(x[b].T),
            "wq": _bf16(wq_), "wk": _bf16(wk_), "wv": _bf16(wv_),
            "projw": _bf16(proj_w[cl0:cl0 + CL, :]),
            "cosT": _bf16(cos2), "sinS": _bf16(sin2), "r2": _bf16(R2),
            "qkbias": np.ascontiguousarray(qkb),
            "bv": _bf16(bv_), "pbh": _bf16(pbh_np),
            "ones": _bf16(ones_np), "masks": _bf16(msk),
        })
    return in_maps


# ====================== PJRT runner (jit once) ======================

_CACHE = {}


def _get_runner():
    if "runner" in _CACHE:
        return _CACHE["runner"]
    import jax
    from jax.sharding import Mesh, PartitionSpec, NamedSharding
    from jax.experimental.shard_map import shard_map
    from concourse import bass2jax

    bass2jax.install_neuronx_cc_hook()
    nc = _build_nc()

    partition_name = (nc.partition_id_tensor.name
                      if nc.partition_id_tensor else None)
    in_names, out_names, out_avals, zero_outs = [], [], [], []
    for alloc in nc.m.functions[0].allocations:
        if not isinstance(alloc, mybir.MemoryLocationSet):
            continue
        name = alloc.memorylocations[0].name
        if alloc.kind == "ExternalInput":
            if name != partition_name:
                in_names.append(name)
        elif alloc.kind == "ExternalOutput":
            shape = tuple(alloc.tensor_shape)
            dtype = mybir.dt.np(alloc.dtype)
            out_names.append(name)
            out_avals.append(jax.core.ShapedArray(shape, dtype))
            zero_outs.append(np.zeros(shape, dtype))
    n_params = len(in_names)
    all_in_names = list(in_names) + list(out_names)
    if partition_name is not None:
        all_in_names.append(partition_name)

    def _body(*args):
        operands = list(args)
        if partition_name is not None:
            operands.append(bass2jax.partition_id_tensor())
        outs = bass2jax._bass_exec_p.bind(
            *operands,
            out_avals=tuple(out_avals),
            in_names=tuple(all_in_names),
            out_names=tuple(out_names),
            lowering_input_output_aliases=(),
            sim_require_finite=True,
            sim_require_nnan=True,
            nc=nc,
        )
        return tuple(outs)

    devices = jax.devices()[:N_CORES]
    mesh = Mesh(np.asarray(devices), ("core",))
    n_outs = len(out_names)
    in_specs = (PartitionSpec("core"),) * (n_params + n_outs)
    out_specs = (PartitionSpec("core"),) * n_outs
    sharded = jax.jit(
        shard_map(_body, mesh=mesh, in_specs=in_specs, out_specs=out_specs,
                  check_rep=False),
        keep_unused=True)

    sh = NamedSharding(mesh, PartitionSpec("core"))

    def prepare(in_maps):
        concat_in = [
            np.concatenate([np.asarray(in_maps[c][nm]) for c in range(N_CORES)],
                           axis=0)
            for nm in in_names
        ]
        concat_zeros = [np.zeros((N_CORES * z.shape[0], *z.shape[1:]), z.dtype)
                        for z in zero_outs]
        import jax as _jax
        return [_jax.device_put(a, sh) for a in concat_in + concat_zeros]

    def run(dev_args):
        outs = sharded(*dev_args)
        import jax as _jax
        _jax.block_until_ready(outs)
        return outs

    def fetch(outs):
        res = []
        arr = np.asarray(outs[0]).reshape(N_CORES, *out_avals[0].shape)
        for c in range(N_CORES):
            res.append({out_names[0]: arr[c]})
        return res

    _CACHE["runner"] = (prepare, run, fetch)
    return _CACHE["runner"]


# ====================== public entry point ======================

def kernel(x, qkv_w, qkv_b, proj_w, proj_b):
    prepare, run, fetch = _get_runner()
    in_maps = _make_in_maps(np.asarray(x), np.asarray(qkv_w),
                            np.asarray(qkv_b), np.asarray(proj_w),
                            np.asarray(proj_b))
    dev_args = prepare(in_maps)
    results = fetch(run(dev_args))
    out = np.zeros((B, T, N_EMBD), dtype=np.float32)
    for b in range(B):
        out[b] = (results[2 * b]["out"].astype(np.float32) +
                  results[2 * b + 1]["out"].astype(np.float32))
    return out


# revision 14
# speedup vs baseline: 1.1593x; 1.1593x over previous
"""Causal self-attention with RoPE on 8 Trainium2 NeuronCores (Bass/Tile).

Self-contained: builds an SPMD Bass kernel, shards the full inputs across the
8 cores as (batch b = core//2, head-half s = core%2), runs via PJRT, and
re-assembles the full [4, 2048, 1024] output (sum of the two proj partials
per batch).

v2: bf16 storage/matmuls (f32 PSUM), flipped PV matmuls (free dim 65),
DMA-engine transposes for the proj input, phase-1 chunks interleaved with
per-qi attention sections so exp (ACT engine) overlaps the GEMMs.

Problem shapes (hardcoded): B=4, T=2048, C=1024, H=16, D=64.
"""
import sys
sys.path.insert(0, "/opt/trn_rl_repo")
import numpy as np
from contextlib import ExitStack

import concourse.bass as bass
import concourse.bacc as bacc
import concourse.mybir as mybir
import concourse.tile as tile

F32 = mybir.dt.float32
BF16 = mybir.dt.bfloat16
AF = mybir.ActivationFunctionType

B = 4
T = 2048
N_EMBD = 1024
D = 64
HL = 8                      # local heads per core
CL = HL * D                 # 512
CT = CL // 128              # 4
VW = HL * 65                # 520 (64 + ones col per head)
CIN_TILES = N_EMBD // 128   # 8
N_CORES = 8
NQT = T // 512              # 4 q blocks (one per phase-1 chunk)
NKT = T // 128              # 16 kv tiles


# ====================== device program ======================

def _build_nc():
    nc = bacc.Bacc("TRN2", target_bir_lowering=False, debug=False,
                   num_devices=N_CORES)

    xT = nc.declare_dram_parameter("xT", [N_EMBD, T], BF16, isOutput=False)
    wq = nc.declare_dram_parameter("wq", [N_EMBD, CL], BF16, isOutput=False)
    wk = nc.declare_dram_parameter("wk", [N_EMBD, CL], BF16, isOutput=False)
    wv = nc.declare_dram_parameter("wv", [N_EMBD, VW], BF16, isOutput=False)
    projw = nc.declare_dram_parameter("projw", [CL, N_EMBD], BF16,
                                      isOutput=False)
    cosT = nc.declare_dram_parameter("cosT", [128, T], BF16, isOutput=False)
    sinS = nc.declare_dram_parameter("sinS", [128, T], BF16, isOutput=False)
    r2 = nc.declare_dram_parameter("r2", [128, 128], BF16, isOutput=False)
    qkbias = nc.declare_dram_parameter("qkbias", [128, 2 * CT], F32,
                                       isOutput=False)
    bv = nc.declare_dram_parameter("bv", [1, VW], BF16, isOutput=False)
    pbh = nc.declare_dram_parameter("pbh", [128, N_EMBD], BF16, isOutput=False)
    ones = nc.declare_dram_parameter("ones", [1, 128], BF16, isOutput=False)
    masks = nc.declare_dram_parameter("masks", [128, 128], BF16,
                                      isOutput=False)
    out = nc.declare_dram_parameter("out", [T, N_EMBD], BF16, isOutput=True)

    with tile.TileContext(nc) as tc, ExitStack() as ctx:
        res = ctx.enter_context(tc.tile_pool(name="res", bufs=1))
        qt_tiles = [res.tile([128, T], BF16, tag=f"qt{i}", name=f"qt{i}")
                    for i in range(CT)]
        kt_tiles = [res.tile([128, T], BF16, tag=f"kt{i}", name=f"kt{i}")
                    for i in range(CT)]
        v_tiles = [res.tile([128, VW], BF16, tag=f"v{j}", name=f"v{j}")
                   for j in range(NKT)]
        yt = res.tile([128, CT * T], BF16, tag="yt")
        wq_sb = res.tile([128, CIN_TILES * CL], BF16, tag="wq")
        wk_sb = res.tile([128, CIN_TILES * CL], BF16, tag="wk")
        wv_sb = res.tile([128, CIN_TILES * VW], BF16, tag="wv")
        pw_sb = res.tile([128, CT * N_EMBD], BF16, tag="pw")
        cos_sb = res.tile([128, T], BF16, tag="cos")
        sin_sb = res.tile([128, T], BF16, tag="sin")
        qkb_sb = res.tile([128, 2 * CT], F32, tag="qkb")
        bvr_sb = res.tile([1, VW], BF16, tag="bvr")
        pbh_sb = res.tile([128, N_EMBD], BF16, tag="pbh")
        ones_sb = res.tile([1, 128], BF16, tag="ones")
        mask_sb = res.tile([128, 128], BF16, tag="masks")
        r2_sb = res.tile([128, 128], BF16, tag="r2")

        # resident DMAs (weights split per cout tile so early deps land first)
        nc.sync.dma_start(ones_sb[:], ones[:])
        nc.sync.dma_start(qkb_sb[:], qkbias[:])
        nc.sync.dma_start(r2_sb[:], r2[:])
        for i in range(CT):
            nc.sync.dma_start(
                wq_sb.rearrange("p (a c) -> p a c", a=CIN_TILES)
                [:, :, i * 128:(i + 1) * 128],
                wq[:, i * 128:(i + 1) * 128]
                .rearrange("(a p) c -> p a c", p=128))
        for i in range(CT):
            nc.sync.dma_start(
                wk_sb.rearrange("p (a c) -> p a c", a=CIN_TILES)
                [:, :, i * 128:(i + 1) * 128],
                wk[:, i * 128:(i + 1) * 128]
                .rearrange("(a p) c -> p a c", p=128))
        nc.sync.dma_start(cos_sb[:], cosT[:])
        nc.sync.dma_start(sin_sb[:], sinS[:])
        nc.sync.dma_start(
            wv_sb.rearrange("p (a c) -> p a c", a=CIN_TILES),
            wv[:].rearrange("(a p) c -> p a c", p=128))
        nc.sync.dma_start(bvr_sb[:], bv[:])
        nc.sync.dma_start(mask_sb[:], masks[:])
        nc.sync.dma_start(
            pw_sb.rearrange("p (a c) -> p a c", a=CT),
            projw[:].rearrange("(a p) c -> p a c", p=128))
        nc.sync.dma_start(pbh_sb[:], pbh[:])

        xpool = ctx.enter_context(tc.tile_pool(name="xpool", bufs=4))
        qbpool = ctx.enter_context(tc.tile_pool(name="qbpool", bufs=3))
        t2pool = ctx.enter_context(tc.tile_pool(name="t2pool", bufs=3))
        ppool = ctx.enter_context(tc.tile_pool(name="ppool", bufs=10))
        ysbp = ctx.enter_context(tc.tile_pool(name="ysbp", bufs=8))
        recp = ctx.enter_context(tc.tile_pool(name="recp", bufs=3))
        otp = ctx.enter_context(tc.tile_pool(name="otp", bufs=4))
        mmps = ctx.enter_context(tc.tile_pool(name="mmps", bufs=2,
                                              space="PSUM"))
        sps = ctx.enter_context(tc.tile_pool(name="sps", bufs=2,
                                             space="PSUM"))
        yqps = ctx.enter_context(tc.tile_pool(name="yqps", bufs=1,
                                              space="PSUM"))
        rotps = ctx.enter_context(tc.tile_pool(name="rotps", bufs=1,
                                               space="PSUM"))

        xc_tiles = {}
        y_sb_cur = {}          # global q-tile idx -> y_sb tile

        def emit_xc(j):
            xc = xpool.tile([128, CIN_TILES * 512], BF16, tag="xc", name="xc")
            tj = slice(j * 512, (j + 1) * 512)
            nc.sync.dma_start(
                xc.rearrange("p (a t) -> p a t", a=CIN_TILES),
                xT[:, tj].rearrange("(a p) t -> p a t", p=128))
            xc_tiles[j] = xc

        def emit_qk_tile(j, which, i):
            """QKV projection + RoPE for one [128, 512] q/k tile."""
            tj = slice(j * 512, (j + 1) * 512)
            xc = xc_tiles[j]
            w_sb = wq_sb if which == 0 else wk_sb
            t_dst = qt_tiles if which == 0 else kt_tiles
            ps = mmps.tile([128, 512], F32, tag="mm", name="qkps")
            for a in range(CIN_TILES):
                nc.tensor.matmul(
                    ps[:],
                    w_sb[:, a * CL + i * 128:a * CL + (i + 1) * 128],
                    xc[:, a * 512:(a + 1) * 512],
                    start=(a == 0), stop=(a == CIN_TILES - 1))
            qb = qbpool.tile([128, 512], BF16, tag="qb", name="qb")
            bcol = which * CT + i
            nc.vector.tensor_scalar_add(qb[:], ps[:],
                                        qkb_sb[:, bcol:bcol + 1])
            dst = t_dst[i][:, tj]
            # rot(qb) via the PE permutation matrix (partition moves must go
            # through the PE; TensorTensor requires same start partitions)
            rps = rotps.tile([128, 512], F32, tag="rot", name="rot")
            nc.tensor.matmul(rps[:], r2_sb[:], qb[:], start=True, stop=True)
            # t1 = qb * cos  (into destination)
            nc.vector.tensor_mul(dst, qb[:], cos_sb[:, tj])
            t2 = t2pool.tile([128, 512], BF16, tag="t2", name="t2")
            nc.vector.tensor_mul(t2[:], rps[:], sin_sb[:, tj])
            nc.vector.tensor_add(dst, dst, t2[:])

        def emit_v_tile(j, tt):
            """V projection for kv tile j*4+tt ([128, 520], time-major)."""
            vt = v_tiles[j * 4 + tt]
            for ch in range(2):
                cw = VW // 2
                vsl = slice(ch * cw, (ch + 1) * cw)
                ps = mmps.tile([128, 512], F32, tag="mm", name="vps")
                for a in range(CIN_TILES):
                    nc.tensor.matmul(
                        ps[:, 0:cw],
                        xc_tiles[j][:, a * 512 + tt * 128:
                                    a * 512 + (tt + 1) * 128],
                        wv_sb[:, a * VW:(a + 1) * VW][:, vsl],
                        start=(a == 0), stop=False)
                nc.tensor.matmul(ps[:, 0:cw], ones_sb[:], bvr_sb[:, vsl],
                                 start=False, stop=True)
                nc.vector.tensor_copy(vt[:, vsl], ps[:, 0:cw])

        def emit_attn_head(qi, h):
            qs = slice(qi * 512, (qi + 1) * 512)
            nk = 4 * (qi + 1)
            th, pb = h // 2, (h % 2) * 64
            kt_h = kt_tiles[th]
            qt_h = qt_tiles[th]
            q0 = qi * 512
            pts = {}
            # ---- S + exp (+ mask) per k-tile pair ----
            for kp in range(nk // 2):
                kb0, kb1 = 2 * kp, 2 * kp + 1
                ks0 = slice(kb0 * 128, (kb0 + 1) * 128)
                ks1 = slice(kb1 * 128, (kb1 + 1) * 128)
                kt0 = kt_h[pb:pb + 64, ks0]
                kt1 = kt_h[pb:pb + 64, ks1]
                mp = kp - 2 * qi
                if mp < 0:                 # fully-causal pair
                    sp = sps.tile([128, 1024], F32, tag="sp", name="sp")
                    nc.tensor.matmul(sp[:, 0:512], kt0, qt_h[pb:pb + 64, qs],
                                     start=True, stop=True)
                    nc.tensor.matmul(sp[:, 512:1024], kt1,
                                     qt_h[pb:pb + 64, qs],
                                     start=True, stop=True)
                    pt = ppool.tile([128, 1024], BF16, tag="pt", name="pt")
                    nc.scalar.activation(pt[:], sp[:], AF.Exp, scale=0.125)
                elif mp == 0:              # first diagonal pair
                    sp = sps.tile([128, 1024], F32, tag="sp", name="sp")
                    nc.tensor.matmul(sp[:, 0:512], kt0, qt_h[pb:pb + 64, qs],
                                     start=True, stop=True)
                    nc.tensor.matmul(sp[:, 512:896], kt1,
                                     qt_h[pb:pb + 64, q0 + 128:q0 + 512],
                                     start=True, stop=True)
                    pt = ppool.tile([128, 1024], BF16, tag="pt", name="pt")
                    nc.scalar.activation(pt[:, 0:896], sp[:, 0:896], AF.Exp,
                                         scale=0.125)
                    # mask only the two diagonal 128x128 blocks (qq=0 at
                    # cols 0:128 of kb0; qq=1 at cols 512:640 of kb1)
                    nc.vector.tensor_mul(pt[:, 0:128], pt[:, 0:128],
                                         mask_sb[:])
                    nc.vector.tensor_mul(pt[:, 512:640], pt[:, 512:640],
                                         mask_sb[:])
                else:                      # last diagonal pair
                    qsub = qt_h[pb:pb + 64, q0 + 256:q0 + 512]
                    sp = sps.tile([128, 1024], F32, tag="sp", name="sp")
                    nc.tensor.matmul(sp[:, 0:256], kt0, qsub,
                                     start=True, stop=True)
                    nc.tensor.matmul(sp[:, 256:512], kt1, qsub,
                                     start=True, stop=True)
                    pt = ppool.tile([128, 1024], BF16, tag="pt", name="pt")
                    nc.scalar.activation(pt[:, 0:512], sp[:, 0:512], AF.Exp,
                                         scale=0.125)
                    # diagonal blocks: qq=2 at cols 0:128 (kb0), qq=3 at
                    # cols 384:512 (kb1)
                    nc.vector.tensor_mul(pt[:, 0:128], pt[:, 0:128],
                                         mask_sb[:])
                    nc.vector.tensor_mul(pt[:, 384:512], pt[:, 384:512],
                                         mask_sb[:])
                pts[kp] = pt

            # pt column offset for (kb, qq): cols of q-subtile within pt
            def pt_cols(kb, qq):
                kp, half = kb // 2, kb % 2
                mp = kp - 2 * qi
                if mp < 0:
                    base = half * 512 + qq * 128
                elif mp == 0:
                    base = qq * 128 if half == 0 else 512 + (qq - 1) * 128
                else:
                    base = (qq - 2) * 128 if half == 0 else 256 + (qq - 2) * 128
                return pts[kp][:, base:base + 128]

            # ---- PV (flipped): yq[q, 65] += pt[kt, q]^T @ v[kt, 65] ----
            yq = yqps.tile([128, 4 * 65], F32, tag="yq", name="yq")
            vsl = slice(h * 65, h * 65 + 65)
            for qq in range(4):
                lo = 0
                hi = 4 * qi + qq            # last kv tile for this q-subtile
                for kb in range(lo, hi + 1):
                    nc.tensor.matmul(yq[:, qq * 65:(qq + 1) * 65],
                                     pt_cols(kb, qq),
                                     v_tiles[kb][:, vsl],
                                     start=(kb == lo), stop=(kb == hi))
            # ---- normalize rows into y_sb (q-major) ----
            rec = recp.tile([128, 4], F32, tag="rec", name="rec")
            for qq in range(4):
                nc.vector.reciprocal(rec[:, qq:qq + 1],
                                     yq[:, qq * 65 + 64:qq * 65 + 65])
            for qq in range(4):
                gq = qi * 4 + qq
                if gq not in y_sb_cur:
                    y_sb_cur[gq] = ysbp.tile([128, 512], BF16, tag="ysb",
                                             name=f"ysb{gq}")
                nc.vector.tensor_scalar_mul(
                    y_sb_cur[gq][:, h * 64:(h + 1) * 64],
                    yq[:, qq * 65:qq * 65 + 64], rec[:, qq:qq + 1])

        def emit_transposes(qi):
            """DMA-engine transposes: y_sb [q, d] -> yt [d, q] per d-chunk."""
            for qq in range(4):
                gq = qi * 4 + qq
                ysb = y_sb_cur[gq]
                for c in range(CT):
                    nc.sync.dma_start_transpose(
                        yt[:, c * T + gq * 128:c * T + (gq + 1) * 128],
                        ysb[:, c * 128:(c + 1) * 128])

        def emit_proj_tt(tt):
            for ch in range(2):
                cs = slice(ch * 512, (ch + 1) * 512)
                ps = mmps.tile([128, 512], F32, tag="mm", name="ops")
                for a in range(CT):
                    nc.tensor.matmul(
                        ps[:],
                        yt[:, a * T + tt * 128:a * T + (tt + 1) * 128],
                        pw_sb[:, a * N_EMBD:(a + 1) * N_EMBD][:, cs],
                        start=(a == 0), stop=(a == CT - 1))
                ot = otp.tile([128, 512], BF16, tag="ot", name="ot")
                nc.vector.tensor_add(ot[:], ps[:], pbh_sb[:, cs])
                nc.sync.dma_start(out[tt * 128:(tt + 1) * 128, cs], ot[:])

        # ================= schedule =================
        for j in range(4):
            emit_xc(j)
        for which in range(2):
            for i in range(CT):
                emit_qk_tile(0, which, i)
        for tt in range(4):
            emit_v_tile(0, tt)

        # sections j=1..3: phase-1 chunk j interleaved with attention qi=j-1
        for j in range(1, 4):
            p1_units = ([(0, i) for i in range(CT)] +
                        [(1, i) for i in range(CT)])
            v_units = list(range(4))
            taken_p1 = taken_v = 0
            for h in range(HL):
                # spread 8 qk + 4 v units across the 8 heads
                want_p1 = ((h + 1) * 8 + 7) // 8
                while taken_p1 < min(want_p1, 8):
                    which, i = p1_units[taken_p1]
                    emit_qk_tile(j, which, i)
                    taken_p1 += 1
                want_v = ((h + 1) * 4) // 8
                while taken_v < want_v:
                    emit_v_tile(j, v_units[taken_v])
                    taken_v += 1
                emit_attn_head(j - 1, h)
            while taken_p1 < 8:
                which, i = p1_units[taken_p1]
                emit_qk_tile(j, which, i)
                taken_p1 += 1
            while taken_v < 4:
                emit_v_tile(j, v_units[taken_v])
                taken_v += 1
            emit_transposes(j - 1)

        # final section: attention qi=3 interleaved with proj of qi=0..2
        proj_units = list(range(12))        # tt 0..11
        taken_pr = 0
        for h in range(HL):
            emit_attn_head(3, h)
            want_pr = ((h + 1) * 12) // 8
            while taken_pr < want_pr:
                emit_proj_tt(proj_units[taken_pr])
                taken_pr += 1
        emit_transposes(3)
        for tt in range(12, 16):
            emit_proj_tt(tt)

    nc.compile()
    return nc


# ====================== host-side sharding ======================

def _rope_tables():
    inv_freq = 1.0 / (10000.0 ** (np.arange(0, D, 2, dtype=np.float64) / D))
    t = np.arange(T, dtype=np.float64)
    fr = np.outer(t, inv_freq)
    emb = np.concatenate([fr, fr], axis=-1)
    return np.cos(emb), np.sin(emb)


def _bf16(a):
    import ml_dtypes
    return np.ascontiguousarray(np.asarray(a).astype(ml_dtypes.bfloat16))


def _make_in_maps(x, qkv_w, qkv_b, proj_w, proj_b):
    C = N_EMBD
    cos, sin = _rope_tables()                   # [T, 64] each
    cos2 = np.vstack([cos.T, cos.T])            # [128, T]
    sin2 = np.vstack([sin.T, sin.T])            # [128, T]
    M = np.zeros((D, D), dtype=np.float32)
    for d in range(32):
        M[d, d + 32] = -1.0
        M[d + 32, d] = 1.0
    R2 = np.zeros((128, 128), dtype=np.float32)
    R2[:64, :64] = M.T
    R2[64:, 64:] = M.T

    p = np.arange(128)[:, None]
    f = np.arange(128)[None, :]
    msk = (p <= f).astype(np.float32)          # lower-tri causal block
    ones_np = np.ones((1, 128), dtype=np.float32)
    pbh_np = np.tile((proj_b / 2.0).astype(np.float32)[None, :], (128, 1))

    in_maps = []
    for c in range(N_CORES):
        b, s = divmod(c, 2)
        cl0 = s * CL
        wq_ = qkv_w[:, cl0:cl0 + CL]
        wk_ = qkv_w[:, C + cl0:C + cl0 + CL]
        wv_raw = qkv_w[:, 2 * C + cl0:2 * C + cl0 + CL]
        bq = qkv_b[cl0:cl0 + CL]
        bk = qkv_b[C + cl0:C + cl0 + CL]
        bv_raw = qkv_b[2 * C + cl0:2 * C + cl0 + CL]
        wv_ = np.zeros((C, VW), dtype=np.float32)
        bv_ = np.zeros((1, VW), dtype=np.float32)
        for h in range(HL):
            wv_[:, 65 * h:65 * h + 64] = wv_raw[:, 64 * h:64 * h + 64]
            bv_[0, 65 * h:65 * h + 64] = bv_raw[64 * h:64 * h + 64]
            bv_[0, 65 * h + 64] = 1.0
        qkb = np.zeros((128, 2 * CT), dtype=np.float32)
        for i in range(CT):
            qkb[:, i] = bq[i * 128:(i + 1) * 128]
            qkb[:, CT + i] = bk[i * 128:(i + 1) * 128]
        in_maps.append({
            "xT": _bf16(x[b].T),
            "wq": _bf16(wq_), "wk": _bf16(wk_), "wv": _bf16(wv_),
            "projw": _bf16(proj_w[cl0:cl0 + CL, :]),
            "cosT": _bf16(cos2), "sinS": _bf16(sin2), "r2": _bf16(R2),
            "qkbias": np.ascontiguousarray(qkb),
            "bv": _bf16(bv_), "pbh": _bf16(pbh_np),
            "ones": _bf16(ones_np), "masks": _bf16(msk),
        })
    return in_maps


# ====================== PJRT runner (jit once) ======================

_CACHE = {}


def _get_runner():
    if "runner" in _CACHE:
        return _CACHE["runner"]
    import jax
    from jax.sharding import Mesh, PartitionSpec, NamedSharding
    from jax.experimental.shard_map import shard_map
    from concourse import bass2jax

    bass2jax.install_neuronx_cc_hook()
    nc = _build_nc()

    partition_name = (nc.partition_id_tensor.name
                      if nc.partition_id_tensor else None)
    in_names, out_names, out_avals, zero_outs = [], [], [], []
    for alloc in nc.m.functions[0].allocations:
        if not isinstance(alloc, mybir.MemoryLocationSet):
            continue
        name = alloc.memorylocations[0].name
        if alloc.kind == "ExternalInput":
            if name != partition_name:
                in_names.append(name)
        elif alloc.kind == "ExternalOutput":
            shape = tuple(alloc.tensor_shape)
            dtype = mybir.dt.np(alloc.dtype)
            out_names.append(name)
            out_avals.append(jax.core.ShapedArray(shape, dtype))
            zero_outs.append(np.zeros(shape, dtype))
    n_params = len(in_names)
    all_in_names = list(in_names) + list(out_names)
    if partition_name is not None:
        all_in_names.append(partition_name)

    def _body(*args):
        operands = list(args)
        if partition_name is not None:
            operands.append(bass2jax.partition_id_tensor())
        outs = bass2jax._bass_exec_p.bind(
            *operands,
            out_avals=tuple(out_avals),
            in_names=tuple(all_in_names),
            out_names=tuple(out_names),
            lowering_input_output_aliases=(),
            sim_require_finite=True,
            sim_require_nnan=True,
            nc=nc,
        )
        return tuple(outs)

    devices = jax.devices()[:N_CORES]
    mesh = Mesh(np.asarray(devices), ("core",))
    n_outs = len(out_names)
    in_specs = (PartitionSpec("core"),) * (n_params + n_outs)
    out_specs = (PartitionSpec("core"),) * n_outs
    sharded = jax.jit(
        shard_map(_body, mesh=mesh, in_specs=in_specs, out_specs=out_specs,
                  check_rep=False),
        keep_unused=True)

    sh = NamedSharding(mesh, PartitionSpec("core"))

    def prepare(in_maps):
        concat_in = [
            np.concatenate([np.asarray(in_maps[c][nm]) for c in range(N_CORES)],
                           axis=0)
            for nm in in_names
        ]
        concat_zeros = [np.zeros((N_CORES * z.shape[0], *z.shape[1:]), z.dtype)
                        for z in zero_outs]
        import jax as _jax
        return [_jax.device_put(a, sh) for a in concat_in + concat_zeros]

    def run(dev_args):
        outs = sharded(*dev_args)
        import jax as _jax
        _jax.block_until_ready(outs)
        return outs

    def fetch(outs):
        res = []
        arr = np.asarray(outs[0]).reshape(N_CORES, *out_avals[0].shape)
        for c in range(N_CORES):
            res.append({out_names[0]: arr[c]})
        return res

    _CACHE["runner"] = (prepare, run, fetch)
    return _CACHE["runner"]


# ====================== public entry point ======================

def kernel(x, qkv_w, qkv_b, proj_w, proj_b):
    prepare, run, fetch = _get_runner()
    in_maps = _make_in_maps(np.asarray(x), np.asarray(qkv_w),
                            np.asarray(qkv_b), np.asarray(proj_w),
                            np.asarray(proj_b))
    dev_args = prepare(in_maps)
    results = fetch(run(dev_args))
    out = np.zeros((B, T, N_EMBD), dtype=np.float32)
    for b in range(B):
        out[b] = (results[2 * b]["out"].astype(np.float32) +
                  results[2 * b + 1]["out"].astype(np.float32))
    return out
